# revision 40
# baseline (speedup 1.0000x reference)

"""Causal attention (no head split) on 8 trn2 NeuronCores.

Reference computation (per batch b):
    q = x @ Wq^T ; k = x @ Wk^T ; v = x @ Wv^T          (nn.Linear convention)
    wei = softmax(mask(q @ k^T / sqrt(C)))               (causal)
    out = wei @ v

Algebraic restructuring (K and V are never materialized):
    S   = q k^T = x (Wq^T Wk) x^T = x M x^T     with M precomputed on host
    out = wei v = (wei x) Wv^T, i.e. O^T = Wv (x^T wei^T) = Wv H
so the device only computes:
    G^T = M^T xq^T                  (projection of this core's queries)
    S^T[s,t] = x^T(lhsT) G^T(rhs)   (contract over C)
    P^T = exp(S^T / 32) * mask ; rowsum[t] += ones^T P^T  (PSUM-accumulated)
    H[c,t] += x(lhsT) P^T(rhs)      (contract over s)
    O^T = Wv^T-projection of H      (once per finished strip pair)
Final softmax normalization (divide by rowsum) happens on the host.

Sharding: 2 cores per batch (B=4); role A owns query rows 512j..512j+256
(strip j), role B rows 512j+256..512j+512.  All device inputs are fp16.
x lives resident in SBUF in both layouts (xT = x^T for S / queries,
xn = x for H).  Per-core the 256-row blocks of x are REORDERED into
slots: slots 0..3 = this core's own query strips (block 2j+role), slots
4..7 = the other role's blocks (2j+1-role).  Under that layout, for both
roles: strip j's queries are xT columns [256j, 256j+256); strip j
attends kv slots {0..j} u {4..4+j}; the diagonal tri mask lands on slot
j and a role-carried data mask (zeros for A / ones for B) on slot 4+j.
One SPMD instruction stream; all role differences are input data.

Strips are processed in PAIRS (0,1) and (2,3): kv slots attended by both
strips of a pair issue N=512 matmuls over the pair's adjacent gT/H
columns (halves instruction count; hides LDWEIGHTS), the pair-younger
strip's extra slots issue N=256.  wm/wv/xT are host-packed so every DMA
lands with >=2KB contiguous runs per partition.
"""
import numpy as np

import concourse.bass as bass
from concourse import bacc
import concourse.mybir as mybir
from concourse.tile import TileContext
from concourse import bass_utils

B, T, C = 4, 2048, 1024
P = 128
CS = C // P          # 8 contraction subtiles
NCH = T // 256       # 8 kv slots of 256
QS = 4               # query strips per core
SW = 256             # strip width
PW = 512             # pair width
SCALE = 1.0 / np.sqrt(C)  # 1/32

F16 = mybir.dt.float16
F32 = mybir.dt.float32

# (slot, wide?) schedule per pair jp: strips j0=2jp (cols 0:256 of the
# pair), j1=2jp+1 (cols 256:512).  Wide slots are attended by both
# strips; narrow slots only by strip j1.  Slot 0 (wide) must come first:
# it initializes hh and the rw PSUM accumulation bank.
def pair_slots(jp):
    j0, j1 = 2 * jp, 2 * jp + 1
    own = [(m, m <= j0) for m in range(j1 + 1)]
    oth = [(4 + k, k <= j0) for k in range(j1 + 1)]
    return own + oth


def build():
    nc = bacc.Bacc(trn_type="TRN2", name="causal_attn")
    # host-packed layouts: every DMA has >=2KB contiguous runs
    xT = nc.dram_tensor("xT", [NCH, P, CS * SW], F16, kind="ExternalInput")
    xn = nc.dram_tensor("xn", [T, C], F16, kind="ExternalInput")
    wm = nc.dram_tensor("wm", [CS, P, C], F16, kind="ExternalInput")
    wv = nc.dram_tensor("wv", [CS, P, C], F16, kind="ExternalInput")
    masks = nc.dram_tensor("masks", [P, 4, SW], F16, kind="ExternalInput")
    outp = nc.dram_tensor("outp", [2, P, CS * PW], F16, kind="ExternalOutput")
    rows = nc.dram_tensor("rows", [1, QS * SW], F32, kind="ExternalOutput")

    xT_r = xT.rearrange("m p (cs t) -> m p cs t", t=SW)
    xn_r = xn.rearrange("(m ss p) c -> p m ss c", p=P, ss=2)
    wm_r = wm.rearrange("ds p (cs d) -> ds p cs d", d=P)
    wv_r = wv.rearrange("ds p (cs d) -> ds p cs d", d=P)
    out_r = outp.rearrange("j p (ds q) -> j p ds q", q=PW)
    out_rf = outp.rearrange("j p (dq q) -> j p dq q", q=SW)

    with TileContext(nc) as tc:
        with tc.tile_pool(name="keep", bufs=1) as keep, \
             tc.tile_pool(name="ppool", bufs=3) as ppool, \
             tc.tile_pool(name="hhpool", bufs=2) as hhpool, \
             tc.tile_pool(name="ostpool", bufs=2) as ostpool, \
             tc.tile_pool(name="psS", bufs=2, space="PSUM") as psS, \
             tc.tile_pool(name="psO", bufs=3, space="PSUM") as psO, \
             tc.tile_pool(name="psR", bufs=1, space="PSUM") as psR:

            xTs = keep.tile([P, CS, T], F16, tag="xTs")       # 32KB/part
            xns = keep.tile([P, NCH, 2, C], F16, tag="xns")   # 32KB/part
            wms = keep.tile([P, CS, C], F16, tag="wms")       # 16KB
            wvs = keep.tile([P, CS, C], F16, tag="wvs")       # 16KB
            gT = keep.tile([P, CS, QS * SW], F16, tag="gT")   # 16KB
            msk = keep.tile([P, 4, SW], F16, tag="msk")
            ones_t = keep.tile([P, P], F16, tag="ones")
            rowsum = keep.tile([1, QS * SW], F32, tag="rowsum")
            warm = keep.tile([P, SW], F16, tag="warm")

            # ---- Warm-up: the first ~12us are DMA-gated (preamble + input
            # transfers).  Spin the PE on dummy matmuls so the HAM clock gate
            # reaches 8/8 before real work arrives.
            nc.vector.memset(warm[:], 0.0)
            nc.vector.memset(ones_t[:], 1.0)
            pw_ps = psO.tile([P, SW], F32, tag="po")
            for _ in range(116):
                nc.tensor.matmul(pw_ps[:, 0:64], warm[:, 0:P], warm[:, 0:64],
                                 start=True, stop=True)

            # ---- DMA schedule.  ALL loads go on the sync(SP) HWDGE ring so
            # the scalar engine stays free for compute (a DMA_DIRECT2D issue
            # stuck on a ring-capacity semaphore blocks everything behind it
            # in that engine's stream).  The ring is FIFO: issue order below
            # IS the arrival priority.
            def ld_xT(m):
                nc.sync.dma_start(xTs[:, :, m * SW:(m + 1) * SW], xT_r[m])

            def ld_wm(ds):
                nc.sync.dma_start(wms[:, :, ds * P:(ds + 1) * P], wm_r[ds])

            def ld_xn(m):
                nc.sync.dma_start(xns[:, m], xn_r[:, m])

            # first transfers split into partition-quarters: 4x DMA-ring
            # parallelism while keeping full 2KB contiguous DRAM runs
            for q in range(4):
                ql = slice(32 * q, 32 * (q + 1))
                nc.sync.dma_start(wms[ql, :, 0:P], wm_r[0, ql])
            for q in range(4):
                ql = slice(32 * q, 32 * (q + 1))
                nc.sync.dma_start(xTs[ql, :, 0:SW], xT_r[0, ql])
            for q in range(4):
                ql = slice(32 * q, 32 * (q + 1))
                nc.sync.dma_start(xTs[ql, :, SW:2 * SW], xT_r[1, ql])
            ld_wm(1); ld_wm(2); ld_wm(3)
            ld_wm(4); ld_wm(5); ld_wm(6); ld_wm(7)
            ld_xT(2); ld_xT(3)
            nc.sync.dma_start(msk[:], masks[:])
            ld_xn(0); ld_xT(4); ld_xn(1); ld_xT(5); ld_xn(4); ld_xn(5)
            for ds in range(CS):
                nc.sync.dma_start(wvs[:, :, ds * P:(ds + 1) * P], wv_r[ds])
            ld_xn(2); ld_xn(3); ld_xT(6); ld_xT(7); ld_xn(6); ld_xn(7)

            # ---- Phase G: G^T = M^T xq^T.  jp-outer, ds-inner.  The
            # first chains run per-strip (N=256) so work starts as soon as
            # wm ds0 + query slot 0 land (slot 1 is still in flight then).
            for jp in range(2):
                for ds in range(CS):
                    dsl = slice(ds * P, (ds + 1) * P)
                    if jp == 0 and ds == 0:
                        for h in range(2):
                            pq = psO.tile([P, SW], F32, tag="po")
                            for cs in range(CS):
                                nc.tensor.matmul(
                                    pq[:], wms[:, cs, dsl],
                                    xTs[:, cs, h * SW:(h + 1) * SW],
                                    start=(cs == 0), stop=(cs == CS - 1))
                            nc.scalar.copy(gT[:, ds, h * SW:(h + 1) * SW], pq[:])
                        continue
                    pq = psO.tile([P, PW], F32, tag="po")
                    for cs in range(CS):
                        nc.tensor.matmul(
                            pq[:], wms[:, cs, dsl],
                            xTs[:, cs, jp * PW:(jp + 1) * PW],
                            start=(cs == 0), stop=(cs == CS - 1))
                    nc.scalar.copy(gT[:, ds, jp * PW:(jp + 1) * PW], pq[:])

            # ---- Pair loop
            for jp in range(2):
                j0, j1 = 2 * jp, 2 * jp + 1
                wide_sl = slice(jp * PW, (jp + 1) * PW)
                narrow_sl = slice(jp * PW + SW, (jp + 1) * PW)
                slots = pair_slots(jp)
                last_m = slots[-1][0]
                rw = psR.tile([P, PW], F32, tag="rw")
                # fp16 H accumulator: half the DVE drain cost of fp32, no
                # cast before the projection (which reads it as matmul rhs).
                # |H| <= ~4k so fp16 accumulation error is ~1e-4 relative.
                hh = hhpool.tile([P, CS, PW], F16, tag="hh")
                for sidx, (m, wide) in enumerate(slots):
                    W = PW if wide else SW
                    gsl = wide_sl if wide else narrow_sl
                    # On the strip's own diagonal slot, the (ss=1, q<128)
                    # quadrant is entirely below the causal diagonal (the tri
                    # mask zeroes it), so S/exp/rowsum/H all skip those 128
                    # columns for ss=1.  pT[:, 1, 0:P] is left stale; no
                    # narrowed consumer reads it.
                    diag = (m == j0) if wide else (m == j1)
                    st = psS.tile([P, 2, W], F32, tag="st")
                    for ss in range(2):
                        o = P if (diag and ss == 1) else 0
                        for cs in range(CS):
                            nc.tensor.matmul(
                                st[:, ss, o:W],
                                xTs[:, cs, m * SW + ss * P:m * SW + (ss + 1) * P],
                                gT[:, cs, gsl.start + o:gsl.stop],
                                start=(cs == 0), stop=(cs == CS - 1))
                    pT = ppool.tile([P, 2, W], F16, tag="pT")
                    if diag:
                        nc.scalar.activation(
                            pT[:, 0], st[:, 0],
                            mybir.ActivationFunctionType.Exp, scale=float(SCALE))
                        nc.scalar.activation(
                            pT[:, 1, P:W], st[:, 1, P:W],
                            mybir.ActivationFunctionType.Exp, scale=float(SCALE))
                    else:
                        nc.scalar.activation(
                            pT[:], st[:],
                            mybir.ActivationFunctionType.Exp, scale=float(SCALE))
                    # diagonal tri mask on the strip's own slot; role-carried
                    # mask (zeros A / ones B) on its other-role slot.
                    if wide:
                        if m == j0:
                            nc.vector.tensor_mul(
                                pT[:, :, 0:SW], pT[:, :, 0:SW], msk[:, 0:2])
                        elif m == 4 + j0:
                            nc.vector.tensor_mul(
                                pT[:, :, 0:SW], pT[:, :, 0:SW], msk[:, 2:4])
                    else:
                        if m == j1:
                            nc.vector.tensor_mul(pT[:], pT[:], msk[:, 0:2])
                        elif m == 4 + j1:
                            nc.vector.tensor_mul(pT[:], pT[:], msk[:, 2:4])

                    # rowsum accumulates in one PSUM bank across the pair.
                    # ones lhsT is [128, 128] (every out row = rowsum): M=1
                    # matmuls pay a ~2x per-instruction floor on hardware.
                    base = 0 if wide else SW
                    for ss in range(2):
                        o = P if (diag and ss == 1) else 0
                        nc.tensor.matmul(
                            rw[:, base + o:base + W], ones_t[:], pT[:, ss, o:W],
                            start=(sidx == 0 and ss == 0),
                            stop=(m == last_m and ss == 1))

                    # H[c, pair cols] += x(lhsT) @ P^T
                    for cs in range(CS):
                        po = psO.tile([P, W], F32, tag="po")
                        if diag:
                            nc.tensor.matmul(
                                po[:], xns[:, m, 0, cs * P:(cs + 1) * P],
                                pT[:, 0], start=True, stop=False,
                                skip_group_check=True)
                            nc.tensor.matmul(
                                po[:, P:W], xns[:, m, 1, cs * P:(cs + 1) * P],
                                pT[:, 1, P:W], start=False, stop=True,
                                skip_group_check=True)
                        else:
                            for ss in range(2):
                                nc.tensor.matmul(
                                    po[:], xns[:, m, ss, cs * P:(cs + 1) * P],
                                    pT[:, ss], start=(ss == 0), stop=(ss == 1))
                        if sidx == 0:
                            nc.vector.tensor_copy(hh[:, cs], po[:])
                        elif wide:
                            nc.vector.tensor_add(hh[:, cs], hh[:, cs], po[:])
                        else:
                            nc.vector.tensor_add(
                                hh[:, cs, SW:PW], hh[:, cs, SW:PW], po[:])


                nc.vector.tensor_copy(rowsum[:, jp * PW:(jp + 1) * PW],
                                      rw[0:1, :])
                nc.sync.dma_start(rows[:, jp * PW:(jp + 1) * PW],
                                  rowsum[:, jp * PW:(jp + 1) * PW])
                ost = ostpool.tile([P, CS, PW], F16, tag="ost")
                for ds in range(CS):
                    pf = psO.tile([P, PW], F32, tag="po")
                    for cs in range(CS):
                        nc.tensor.matmul(
                            pf[:], wvs[:, cs, ds * P:(ds + 1) * P], hh[:, cs],
                            start=(cs == 0), stop=(cs == CS - 1))
                    # alternate copy engines.  jp0 stores go as ds pairs
                    # (plenty of slack); jp1 stores stream out in 256-col
                    # chunks across many DMA rings so the kernel end is not
                    # single-ring transfer bound.
                    ost_rf = ost.rearrange("p ds (c q) -> p (ds c) q", q=SW)
                    if ds % 2 == 0:
                        nc.scalar.copy(ost[:, ds], pf[:])
                        if jp == 1 and ds == 6:
                            for c in range(2):
                                nc.sync.dma_start(
                                    out_rf[jp, :, 12 + c:13 + c],
                                    ost_rf[:, 12 + c:13 + c])
                    else:
                        if jp == 1 and ds == 7:
                            nc.scalar.copy(ost[:, ds, 0:SW], pf[:, 0:SW])
                            nc.sync.dma_start(out_rf[jp, :, 14:15],
                                              ost_rf[:, 14:15])
                            nc.vector.tensor_copy(ost[:, ds, SW:PW], pf[:, SW:PW])
                            nc.sync.dma_start(out_rf[jp, :, 15:16],
                                              ost_rf[:, 15:16])
                        else:
                            nc.vector.tensor_copy(ost[:, ds], pf[:])
                            if jp == 0:
                                nc.sync.dma_start(out_r[jp, :, ds - 1:ds + 1],
                                                  ost[:, ds - 1:ds + 1])
                            else:
                                for c in range(4):
                                    nc.sync.dma_start(
                                        out_rf[jp, :, 2 * ds - 2 + c:2 * ds - 1 + c],
                                        ost_rf[:, 2 * ds - 2 + c:2 * ds - 1 + c])

    nc.compile()
    return nc


_NC = None


def _get_nc():
    global _NC
    if _NC is None:
        _NC = build()
    return _NC


# per-role slot order of 256-row blocks: own strips first, then the
# other role's blocks
_ORDER_A = np.array([0, 2, 4, 6, 1, 3, 5, 7])
_ORDER_B = np.array([1, 3, 5, 7, 0, 2, 4, 6])


def make_in_maps(x, Wq, Wk, Wv):
    x = np.asarray(x, dtype=np.float32)
    wq64 = np.asarray(Wq, np.float64)
    wk64 = np.asarray(Wk, np.float64)
    M = (wq64.T @ wk64).astype(np.float16)                   # [c, d]
    WvT = np.asarray(Wv, np.float32).T.astype(np.float16)    # [c, d]
    # pack [c, d] -> [ds, p, cs*128+d2] so per-partition runs are 2KB
    def packw(w):
        return np.ascontiguousarray(
            w.reshape(CS, P, CS, P).transpose(2, 1, 0, 3).reshape(CS, P, C))
    wmp, wvp = packw(M), packw(WvT)

    # tri[p, ss, t] = 1 if (ss*128+p) <= t   (diagonal chunk mask)
    s_idx = (np.arange(2)[None, :, None] * P + np.arange(P)[:, None, None])
    tri = (s_idx <= np.arange(SW)[None, None, :]).astype(np.float16)
    zer = np.zeros((P, 2, SW), np.float16)
    one2 = np.ones((P, 2, SW), np.float16)
    mask_A = np.ascontiguousarray(np.concatenate([tri, zer], axis=1))
    mask_B = np.ascontiguousarray(np.concatenate([tri, one2], axis=1))

    in_maps = []
    for core in range(8):
        b, role = divmod(core, 2)
        order = _ORDER_A if role == 0 else _ORDER_B
        xh = x[b].astype(np.float16).reshape(NCH, SW, C)[order]  # [slot,256,C]
        xn_in = np.ascontiguousarray(xh.reshape(T, C))
        xTp = np.ascontiguousarray(
            xh.reshape(NCH, SW, CS, P).transpose(0, 3, 2, 1)     # [m,p,cs,t2]
            .reshape(NCH, P, CS * SW))
        in_maps.append({
            "xT": xTp,
            "xn": xn_in,
            "wm": wmp, "wv": wvp,
            "masks": mask_A if role == 0 else mask_B,
        })
    return in_maps


def assemble(results):
    out = np.empty((B, T, C), np.float32)
    for core in range(8):
        b, role = divmod(core, 2)
        op = results[core]["outp"].astype(np.float32)
        oT = op.reshape(2, P, CS, PW).transpose(2, 1, 0, 3).reshape(C, QS * SW)
        rsum = results[core]["rows"].reshape(QS * SW)
        o = oT.T / rsum[:, None]                             # [1024 q, C]
        for j in range(QS):
            r0 = 512 * j + SW * role
            out[b, r0:r0 + SW] = o[j * SW:(j + 1) * SW]
    return out


def kernel(x, Wq, Wk, Wv):
    nc = _get_nc()
    in_maps = make_in_maps(x, Wq, Wk, Wv)
    res = bass_utils.run_bass_kernel_spmd(nc, in_maps, core_ids=list(range(8)))
    return assemble(res.results)


def _install_trace_shim():
    """Provide antenv.axon_hooks (absent in this image) so trace=True works."""
    import sys
    import types
    if "antenv.axon_hooks" in sys.modules:
        return
    hook_box = [None]
    mod = types.ModuleType("antenv.axon_hooks")
    mod.set_axon_ntff_profile_hook = lambda h: hook_box.__setitem__(0, h)
    mod.get_axon_ntff_profile_hook = lambda: hook_box[0]
    import antenv
    sys.modules["antenv.axon_hooks"] = mod
    antenv.axon_hooks = mod
    try:
        from trn_agent_boot.trn_boot import _ntff_profile_via_ctypes
        mod.set_axon_ntff_profile_hook(
            _ntff_profile_via_ctypes("/opt/axon/libaxon_pjrt.so"))
    except Exception:
        pass


def run_traced(x, Wq, Wk, Wv):
    """Like kernel() but with NTFF tracing; returns (out, BassKernelResults)."""
    _install_trace_shim()
    nc = _get_nc()
    in_maps = make_in_maps(x, Wq, Wk, Wv)
    res = bass_utils.run_bass_kernel_spmd(
        nc, in_maps, core_ids=list(range(8)), trace=True,
        trace_cores=list(range(8)))
    return assemble(res.results), res



# revision 41
# speedup vs baseline: 1.0200x; 1.0200x over previous

"""Causal attention (no head split) on 8 trn2 NeuronCores.

Reference computation (per batch b):
    q = x @ Wq^T ; k = x @ Wk^T ; v = x @ Wv^T          (nn.Linear convention)
    wei = softmax(mask(q @ k^T / sqrt(C)))               (causal)
    out = wei @ v

Algebraic restructuring (K and V are never materialized):
    S   = q k^T = x (Wq^T Wk) x^T = x M x^T     with M precomputed on host
    out = wei v = (wei x) Wv^T, i.e. O^T = Wv (x^T wei^T) = Wv H
so the device only computes:
    G^T = M^T xq^T                  (projection of this core's queries)
    S^T[s,t] = x^T(lhsT) G^T(rhs)   (contract over C)
    P^T = exp(S^T / 32) * mask ; rowsum[t] += ones^T P^T  (PSUM-accumulated)
    H[c,t] += x(lhsT) P^T(rhs)      (contract over s)
    O^T = Wv^T-projection of H      (once per finished strip pair)
Final softmax normalization (divide by rowsum) happens on the host.

Sharding: 2 cores per batch (B=4); role A owns query rows 512j..512j+256
(strip j), role B rows 512j+256..512j+512.  All device inputs are fp16.
x lives resident in SBUF in both layouts (xT = x^T for S / queries,
xn = x for H).  Per-core the 256-row blocks of x are REORDERED into
slots: slots 0..3 = this core's own query strips (block 2j+role), slots
4..7 = the other role's blocks (2j+1-role).  Under that layout, for both
roles: strip j's queries are xT columns [256j, 256j+256); strip j
attends kv slots {0..j} u {4..4+j}; the diagonal tri mask lands on slot
j and a role-carried data mask (zeros for A / ones for B) on slot 4+j.
One SPMD instruction stream; all role differences are input data.

Strips are processed in PAIRS (0,1) and (2,3): kv slots attended by both
strips of a pair issue N=512 matmuls over the pair's adjacent gT/H
columns (halves instruction count; hides LDWEIGHTS), the pair-younger
strip's extra slots issue N=256.  wm/wv/xT are host-packed so every DMA
lands with >=2KB contiguous runs per partition.
"""
import numpy as np

import concourse.bass as bass
from concourse import bacc
import concourse.mybir as mybir
from concourse.tile import TileContext
from concourse import bass_utils

B, T, C = 4, 2048, 1024
P = 128
CS = C // P          # 8 contraction subtiles
NCH = T // 256       # 8 kv slots of 256
QS = 4               # query strips per core
SW = 256             # strip width
PW = 512             # pair width
SCALE = 1.0 / np.sqrt(C)  # 1/32

F16 = mybir.dt.float16
F32 = mybir.dt.float32

# (slot, wide?) schedule per pair jp: strips j0=2jp (cols 0:256 of the
# pair), j1=2jp+1 (cols 256:512).  Wide slots are attended by both
# strips; narrow slots only by strip j1.  Slot 0 (wide) must come first:
# it initializes hh and the rw PSUM accumulation bank.
def pair_slots(jp):
    j0, j1 = 2 * jp, 2 * jp + 1
    own = [(m, m <= j0) for m in range(j1 + 1)]
    oth = [(4 + k, k <= j0) for k in range(j1 + 1)]
    return own + oth


def build():
    nc = bacc.Bacc(trn_type="TRN2", name="causal_attn")
    # host-packed layouts: every DMA has >=2KB contiguous runs
    xT = nc.dram_tensor("xT", [NCH, P, CS * SW], F16, kind="ExternalInput")
    xn = nc.dram_tensor("xn", [T, C], F16, kind="ExternalInput")
    wm = nc.dram_tensor("wm", [CS, P, C], F16, kind="ExternalInput")
    wv = nc.dram_tensor("wv", [CS, P, C], F16, kind="ExternalInput")
    masks = nc.dram_tensor("masks", [P, 4, SW], F16, kind="ExternalInput")
    outp = nc.dram_tensor("outp", [2, P, CS * PW], F16, kind="ExternalOutput")
    rows = nc.dram_tensor("rows", [1, QS * SW], F32, kind="ExternalOutput")

    xT_r = xT.rearrange("m p (cs t) -> m p cs t", t=SW)
    xn_r = xn.rearrange("(m ss p) c -> p m ss c", p=P, ss=2)
    wm_r = wm.rearrange("ds p (cs d) -> ds p cs d", d=P)
    wv_r = wv.rearrange("ds p (cs d) -> ds p cs d", d=P)
    out_r = outp.rearrange("j p (ds q) -> j p ds q", q=PW)
    out_rf = outp.rearrange("j p (dq q) -> j p dq q", q=SW)

    with TileContext(nc) as tc:
        with tc.tile_pool(name="keep", bufs=1) as keep, \
             tc.tile_pool(name="ppool", bufs=3) as ppool, \
             tc.tile_pool(name="hhpool", bufs=2) as hhpool, \
             tc.tile_pool(name="ostpool", bufs=2) as ostpool, \
             tc.tile_pool(name="psS", bufs=2, space="PSUM") as psS, \
             tc.tile_pool(name="psO", bufs=3, space="PSUM") as psO, \
             tc.tile_pool(name="psR", bufs=1, space="PSUM") as psR:

            xTs = keep.tile([P, CS, T], F16, tag="xTs")       # 32KB/part
            xns = keep.tile([P, NCH, 2, C], F16, tag="xns")   # 32KB/part
            wms = keep.tile([P, CS, C], F16, tag="wms")       # 16KB
            wvs = keep.tile([P, CS, C], F16, tag="wvs")       # 16KB
            gT = keep.tile([P, CS, QS * SW], F16, tag="gT")   # 16KB
            msk = keep.tile([P, 4, SW], F16, tag="msk")
            ones_t = keep.tile([P, P], F16, tag="ones")
            rowsum = keep.tile([1, QS * SW], F32, tag="rowsum")
            warm = keep.tile([P, SW], F16, tag="warm")

            # ---- Warm-up: the first ~12us are DMA-gated (preamble + input
            # transfers).  Spin the PE on dummy matmuls so the HAM clock gate
            # reaches 8/8 before real work arrives.
            nc.vector.memset(warm[:], 0.0)
            nc.vector.memset(ones_t[:], 1.0)
            pw_ps = psO.tile([P, SW], F32, tag="po")
            for _ in range(64):
                nc.tensor.matmul(pw_ps[:, 0:64], warm[:, 0:P], warm[:, 0:64],
                                 start=True, stop=True)

            # ---- DMA schedule.  ALL loads go on the sync(SP) HWDGE ring so
            # the scalar engine stays free for compute (a DMA_DIRECT2D issue
            # stuck on a ring-capacity semaphore blocks everything behind it
            # in that engine's stream).  The ring is FIFO: issue order below
            # IS the arrival priority.
            def ld_xT(m):
                nc.sync.dma_start(xTs[:, :, m * SW:(m + 1) * SW], xT_r[m])

            def ld_wm(ds):
                nc.sync.dma_start(wms[:, :, ds * P:(ds + 1) * P], wm_r[ds])

            def ld_xn(m):
                nc.sync.dma_start(xns[:, m], xn_r[:, m])

            # first transfers split in halves for more DMA-ring parallelism
            nc.sync.dma_start(wms[:, 0:4, 0:P], wm_r[0, :, 0:4])
            nc.sync.dma_start(wms[:, 4:8, 0:P], wm_r[0, :, 4:8])
            nc.sync.dma_start(xTs[:, 0:4, 0:SW], xT_r[0, :, 0:4])
            nc.sync.dma_start(xTs[:, 4:8, 0:SW], xT_r[0, :, 4:8])
            nc.sync.dma_start(xTs[:, 0:4, SW:2 * SW], xT_r[1, :, 0:4])
            nc.sync.dma_start(xTs[:, 4:8, SW:2 * SW], xT_r[1, :, 4:8])
            ld_wm(1); ld_wm(2); ld_wm(3)
            ld_wm(4); ld_wm(5); ld_wm(6); ld_wm(7)
            ld_xT(2); ld_xT(3)
            nc.sync.dma_start(msk[:], masks[:])
            ld_xn(0); ld_xT(4); ld_xn(1); ld_xT(5); ld_xn(4); ld_xn(5)
            for ds in range(CS):
                nc.sync.dma_start(wvs[:, :, ds * P:(ds + 1) * P], wv_r[ds])
            ld_xn(2); ld_xn(3); ld_xT(6); ld_xT(7); ld_xn(6); ld_xn(7)

            # ---- Phase G: G^T = M^T xq^T.  jp-outer, ds-inner.  The
            # first chains run per-strip (N=256) so work starts as soon as
            # wm ds0 + query slot 0 land (slot 1 is still in flight then).
            for jp in range(2):
                for ds in range(CS):
                    dsl = slice(ds * P, (ds + 1) * P)
                    if jp == 0 and ds == 0:
                        for h in range(2):
                            pq = psO.tile([P, SW], F32, tag="po")
                            for cs in range(CS):
                                nc.tensor.matmul(
                                    pq[:], wms[:, cs, dsl],
                                    xTs[:, cs, h * SW:(h + 1) * SW],
                                    start=(cs == 0), stop=(cs == CS - 1))
                            nc.scalar.copy(gT[:, ds, h * SW:(h + 1) * SW], pq[:])
                        continue
                    pq = psO.tile([P, PW], F32, tag="po")
                    for cs in range(CS):
                        nc.tensor.matmul(
                            pq[:], wms[:, cs, dsl],
                            xTs[:, cs, jp * PW:(jp + 1) * PW],
                            start=(cs == 0), stop=(cs == CS - 1))
                    nc.scalar.copy(gT[:, ds, jp * PW:(jp + 1) * PW], pq[:])

            # ---- Pair loop
            for jp in range(2):
                j0, j1 = 2 * jp, 2 * jp + 1
                wide_sl = slice(jp * PW, (jp + 1) * PW)
                narrow_sl = slice(jp * PW + SW, (jp + 1) * PW)
                slots = pair_slots(jp)
                last_m = slots[-1][0]
                rw = psR.tile([P, PW], F32, tag="rw")
                # fp16 H accumulator: half the DVE drain cost of fp32, no
                # cast before the projection (which reads it as matmul rhs).
                # |H| <= ~4k so fp16 accumulation error is ~1e-4 relative.
                hh = hhpool.tile([P, CS, PW], F16, tag="hh")
                for sidx, (m, wide) in enumerate(slots):
                    W = PW if wide else SW
                    gsl = wide_sl if wide else narrow_sl
                    # On the strip's own diagonal slot, the (ss=1, q<128)
                    # quadrant is entirely below the causal diagonal (the tri
                    # mask zeroes it), so S/exp/rowsum/H all skip those 128
                    # columns for ss=1.  pT[:, 1, 0:P] is left stale; no
                    # narrowed consumer reads it.
                    diag = (m == j0) if wide else (m == j1)
                    st = psS.tile([P, 2, W], F32, tag="st")
                    for ss in range(2):
                        o = P if (diag and ss == 1) else 0
                        for cs in range(CS):
                            nc.tensor.matmul(
                                st[:, ss, o:W],
                                xTs[:, cs, m * SW + ss * P:m * SW + (ss + 1) * P],
                                gT[:, cs, gsl.start + o:gsl.stop],
                                start=(cs == 0), stop=(cs == CS - 1))
                    pT = ppool.tile([P, 2, W], F16, tag="pT")
                    if diag:
                        nc.scalar.activation(
                            pT[:, 0], st[:, 0],
                            mybir.ActivationFunctionType.Exp, scale=float(SCALE))
                        nc.scalar.activation(
                            pT[:, 1, P:W], st[:, 1, P:W],
                            mybir.ActivationFunctionType.Exp, scale=float(SCALE))
                    else:
                        nc.scalar.activation(
                            pT[:], st[:],
                            mybir.ActivationFunctionType.Exp, scale=float(SCALE))
                    # diagonal tri mask on the strip's own slot; role-carried
                    # mask (zeros A / ones B) on its other-role slot.
                    if wide:
                        if m == j0:
                            nc.vector.tensor_mul(
                                pT[:, :, 0:SW], pT[:, :, 0:SW], msk[:, 0:2])
                        elif m == 4 + j0:
                            nc.vector.tensor_mul(
                                pT[:, :, 0:SW], pT[:, :, 0:SW], msk[:, 2:4])
                    else:
                        if m == j1:
                            nc.vector.tensor_mul(pT[:], pT[:], msk[:, 0:2])
                        elif m == 4 + j1:
                            nc.vector.tensor_mul(pT[:], pT[:], msk[:, 2:4])

                    # rowsum accumulates in one PSUM bank across the pair.
                    # ones lhsT is [128, 128] (every out row = rowsum): M=1
                    # matmuls pay a ~2x per-instruction floor on hardware.
                    base = 0 if wide else SW
                    for ss in range(2):
                        o = P if (diag and ss == 1) else 0
                        nc.tensor.matmul(
                            rw[:, base + o:base + W], ones_t[:], pT[:, ss, o:W],
                            start=(sidx == 0 and ss == 0),
                            stop=(m == last_m and ss == 1))

                    # H[c, pair cols] += x(lhsT) @ P^T
                    for cs in range(CS):
                        po = psO.tile([P, W], F32, tag="po")
                        if diag:
                            nc.tensor.matmul(
                                po[:], xns[:, m, 0, cs * P:(cs + 1) * P],
                                pT[:, 0], start=True, stop=False,
                                skip_group_check=True)
                            nc.tensor.matmul(
                                po[:, P:W], xns[:, m, 1, cs * P:(cs + 1) * P],
                                pT[:, 1, P:W], start=False, stop=True,
                                skip_group_check=True)
                        else:
                            for ss in range(2):
                                nc.tensor.matmul(
                                    po[:], xns[:, m, ss, cs * P:(cs + 1) * P],
                                    pT[:, ss], start=(ss == 0), stop=(ss == 1))
                        if sidx == 0:
                            nc.vector.tensor_copy(hh[:, cs], po[:])
                        elif wide:
                            nc.vector.tensor_add(hh[:, cs], hh[:, cs], po[:])
                        else:
                            nc.vector.tensor_add(
                                hh[:, cs, SW:PW], hh[:, cs, SW:PW], po[:])


                nc.vector.tensor_copy(rowsum[:, jp * PW:(jp + 1) * PW],
                                      rw[0:1, :])
                nc.sync.dma_start(rows[:, jp * PW:(jp + 1) * PW],
                                  rowsum[:, jp * PW:(jp + 1) * PW])
                ost = ostpool.tile([P, CS, PW], F16, tag="ost")
                for ds in range(CS):
                    pf = psO.tile([P, PW], F32, tag="po")
                    for cs in range(CS):
                        nc.tensor.matmul(
                            pf[:], wvs[:, cs, ds * P:(ds + 1) * P], hh[:, cs],
                            start=(cs == 0), stop=(cs == CS - 1))
                    # alternate copy engines.  jp0 stores go as ds pairs
                    # (plenty of slack); jp1 stores stream out in 256-col
                    # chunks across many DMA rings so the kernel end is not
                    # single-ring transfer bound.
                    ost_rf = ost.rearrange("p ds (c q) -> p (ds c) q", q=SW)
                    if ds % 2 == 0:
                        nc.scalar.copy(ost[:, ds], pf[:])
                        if jp == 1 and ds == 6:
                            for c in range(2):
                                nc.sync.dma_start(
                                    out_rf[jp, :, 12 + c:13 + c],
                                    ost_rf[:, 12 + c:13 + c])
                    else:
                        if jp == 1 and ds == 7:
                            nc.scalar.copy(ost[:, ds, 0:SW], pf[:, 0:SW])
                            nc.sync.dma_start(out_rf[jp, :, 14:15],
                                              ost_rf[:, 14:15])
                            nc.vector.tensor_copy(ost[:, ds, SW:PW], pf[:, SW:PW])
                            nc.sync.dma_start(out_rf[jp, :, 15:16],
                                              ost_rf[:, 15:16])
                        else:
                            nc.vector.tensor_copy(ost[:, ds], pf[:])
                            if jp == 0:
                                nc.sync.dma_start(out_r[jp, :, ds - 1:ds + 1],
                                                  ost[:, ds - 1:ds + 1])
                            else:
                                for c in range(4):
                                    nc.sync.dma_start(
                                        out_rf[jp, :, 2 * ds - 2 + c:2 * ds - 1 + c],
                                        ost_rf[:, 2 * ds - 2 + c:2 * ds - 1 + c])

    nc.compile()
    return nc


_NC = None


def _get_nc():
    global _NC
    if _NC is None:
        _NC = build()
    return _NC


# per-role slot order of 256-row blocks: own strips first, then the
# other role's blocks
_ORDER_A = np.array([0, 2, 4, 6, 1, 3, 5, 7])
_ORDER_B = np.array([1, 3, 5, 7, 0, 2, 4, 6])


def make_in_maps(x, Wq, Wk, Wv):
    x = np.asarray(x, dtype=np.float32)
    wq64 = np.asarray(Wq, np.float64)
    wk64 = np.asarray(Wk, np.float64)
    M = (wq64.T @ wk64).astype(np.float16)                   # [c, d]
    WvT = np.asarray(Wv, np.float32).T.astype(np.float16)    # [c, d]
    # pack [c, d] -> [ds, p, cs*128+d2] so per-partition runs are 2KB
    def packw(w):
        return np.ascontiguousarray(
            w.reshape(CS, P, CS, P).transpose(2, 1, 0, 3).reshape(CS, P, C))
    wmp, wvp = packw(M), packw(WvT)

    # tri[p, ss, t] = 1 if (ss*128+p) <= t   (diagonal chunk mask)
    s_idx = (np.arange(2)[None, :, None] * P + np.arange(P)[:, None, None])
    tri = (s_idx <= np.arange(SW)[None, None, :]).astype(np.float16)
    zer = np.zeros((P, 2, SW), np.float16)
    one2 = np.ones((P, 2, SW), np.float16)
    mask_A = np.ascontiguousarray(np.concatenate([tri, zer], axis=1))
    mask_B = np.ascontiguousarray(np.concatenate([tri, one2], axis=1))

    in_maps = []
    for core in range(8):
        b, role = divmod(core, 2)
        order = _ORDER_A if role == 0 else _ORDER_B
        xh = x[b].astype(np.float16).reshape(NCH, SW, C)[order]  # [slot,256,C]
        xn_in = np.ascontiguousarray(xh.reshape(T, C))
        xTp = np.ascontiguousarray(
            xh.reshape(NCH, SW, CS, P).transpose(0, 3, 2, 1)     # [m,p,cs,t2]
            .reshape(NCH, P, CS * SW))
        in_maps.append({
            "xT": xTp,
            "xn": xn_in,
            "wm": wmp, "wv": wvp,
            "masks": mask_A if role == 0 else mask_B,
        })
    return in_maps


def assemble(results):
    out = np.empty((B, T, C), np.float32)
    for core in range(8):
        b, role = divmod(core, 2)
        op = results[core]["outp"].astype(np.float32)
        oT = op.reshape(2, P, CS, PW).transpose(2, 1, 0, 3).reshape(C, QS * SW)
        rsum = results[core]["rows"].reshape(QS * SW)
        o = oT.T / rsum[:, None]                             # [1024 q, C]
        for j in range(QS):
            r0 = 512 * j + SW * role
            out[b, r0:r0 + SW] = o[j * SW:(j + 1) * SW]
    return out


def kernel(x, Wq, Wk, Wv):
    nc = _get_nc()
    in_maps = make_in_maps(x, Wq, Wk, Wv)
    res = bass_utils.run_bass_kernel_spmd(nc, in_maps, core_ids=list(range(8)))
    return assemble(res.results)


def _install_trace_shim():
    """Provide antenv.axon_hooks (absent in this image) so trace=True works."""
    import sys
    import types
    if "antenv.axon_hooks" in sys.modules:
        return
    hook_box = [None]
    mod = types.ModuleType("antenv.axon_hooks")
    mod.set_axon_ntff_profile_hook = lambda h: hook_box.__setitem__(0, h)
    mod.get_axon_ntff_profile_hook = lambda: hook_box[0]
    import antenv
    sys.modules["antenv.axon_hooks"] = mod
    antenv.axon_hooks = mod
    try:
        from trn_agent_boot.trn_boot import _ntff_profile_via_ctypes
        mod.set_axon_ntff_profile_hook(
            _ntff_profile_via_ctypes("/opt/axon/libaxon_pjrt.so"))
    except Exception:
        pass


def run_traced(x, Wq, Wk, Wv):
    """Like kernel() but with NTFF tracing; returns (out, BassKernelResults)."""
    _install_trace_shim()
    nc = _get_nc()
    in_maps = make_in_maps(x, Wq, Wk, Wv)
    res = bass_utils.run_bass_kernel_spmd(
        nc, in_maps, core_ids=list(range(8)), trace=True,
        trace_cores=list(range(8)))
    return assemble(res.results), res

